# revision 1
# baseline (speedup 1.0000x reference)
"""Multi-head attention (B=4, S=2048, D=1024, H=16) on 8 Trainium2 cores.

Sharding: core c handles batch b = c//2 and head-group hg = c%2 (8 of the 16
heads, i.e. 512 of the 1024 projection dims).  Every core computes:

    Qc^T = (Wq_cols^T @ q[b]^T)           [512, 2048]   (proj-major layout)
    Kc^T = (Wk_cols^T @ k[b]^T)           [512, 2048]
    Vc   = (v[b] @ Wv_cols)               [2048, 512]
    S^T  = Kc_h @ Qc_h^T per head         (scores, transposed: [keys, queries])
    P^T  = exp(S^T/8 + maskbias)          (ACT engine, fused scale+mask)
    A^T  = V_h^T @ P^T   and  l = 1^T P^T (AV + denominator via matmul)
    A^T  = A^T * (1/l)                    (broadcast via selector matmul)
    out_partial = A_c @ Wo_rows           [2048, 1024]

Host sums the two partial outputs per batch (the "all-reduce after w_o")
and adds the folded bias bv @ Wo + bo.  Biases bq/bk are applied on-device
(per-partition adds); the mask is applied as an additive bias inside the
exp activation.

All matmuls run as float32r (fp32 storage, single-pass PE mode).
"""

import os
import numpy as np

B, S, D = 4, 2048, 1024
H, DK = 16, 64
P = 128
NCORES = 8
HPC = H // 2            # heads per core
PROJ = HPC * DK         # 512 projection dims per core
NDM = D // P            # 8 d_model chunks
NPC = PROJ // P         # 4 head-pair chunks
NSC = S // 512          # 4 seq chunks of 512
NSO = S // P            # 16 seq chunks of 128
NKC = S // P            # 16 key chunks of 128

MASK_NEG = -30000.0     # exp(x - 30000) == 0 in fp32 for any plausible x

_cache = {}


def _build():
    """Build + compile the per-core Bass program (same program on all cores)."""
    import concourse.bass as bass
    import concourse.bacc as bacc
    import concourse.mybir as mybir
    import concourse.tile as tile
    from contextlib import ExitStack

    f32 = mybir.dt.float32
    f32r = mybir.dt.float32r
    bf16 = mybir.dt.bfloat16
    AF = mybir.ActivationFunctionType
    MUL = mybir.AluOpType.mult

    nc = bacc.Bacc("TRN2", target_bir_lowering=False, debug=False,
                   num_devices=NCORES)

    qT = nc.dram_tensor("qT", [D, S], bf16, kind="ExternalInput").ap()
    kT = nc.dram_tensor("kT", [D, S], bf16, kind="ExternalInput").ap()
    vT = nc.dram_tensor("vT", [D, S], bf16, kind="ExternalInput").ap()
    wq = nc.dram_tensor("wq", [D, PROJ], bf16, kind="ExternalInput").ap()
    wk = nc.dram_tensor("wk", [D, PROJ], bf16, kind="ExternalInput").ap()
    wv = nc.dram_tensor("wv", [D, PROJ], bf16, kind="ExternalInput").ap()
    wo = nc.dram_tensor("wo", [PROJ, D], bf16, kind="ExternalInput").ap()
    bq2 = nc.dram_tensor("bq2", [P, NPC], f32, kind="ExternalInput").ap()
    bk2 = nc.dram_tensor("bk2", [P, NPC], f32, kind="ExternalInput").ap()
    mb = nc.dram_tensor("mb", [P, NKC], f32, kind="ExternalInput").ap()
    sel = nc.dram_tensor("sel", [P, 128], f32, kind="ExternalInput").ap()
    vones = nc.dram_tensor("vones", [P, NSO, HPC], bf16, kind="ExternalInput").ap()
    out = nc.dram_tensor("out", [S, D], f32, kind="ExternalOutput").ap()

    def r(x):
        return x

    with tile.TileContext(nc) as tc, ExitStack() as ctx:
        cpool = ctx.enter_context(tc.tile_pool(name="const", bufs=1))
        sel_sb = cpool.tile([P, 128], f32)
        nc.sync.dma_start(sel_sb[:], sel)
        mb_sb = cpool.tile([P, NKC], f32)
        nc.sync.dma_start(mb_sb[:], mb)
        bq_sb = cpool.tile([P, NPC], f32)
        nc.sync.dma_start(bq_sb[:], bq2)
        bk_sb = cpool.tile([P, NPC], f32)
        nc.sync.dma_start(bk_sb[:], bk2)

        wopool = ctx.enter_context(tc.tile_pool(name="wo", bufs=1))
        wo_sb = wopool.tile([P, NPC, D], bf16)
        nc.sync.dma_start(wo_sb[:], wo.rearrange("(o p) n -> p o n", p=P))

        # Long-lived SBUF pools allocated up front so later-phase tiles never
        # reuse phase-A addresses (address reuse adds false serialization).
        respool = ctx.enter_context(tc.tile_pool(name="res", bufs=1))
        QT_sb = respool.tile([P, NPC, S], bf16)   # [pair-chunk, seq]
        # K^T stored per head on the full 128-partition contraction range:
        # even heads carry data in rows 0-63 (rows 64-127 zero), odd heads
        # in rows 64-127 (rows 0-63 zero).  The S^T matmul is then a
        # standard-mode 128x128 matmul against the pair-stacked Q^T -- no
        # PE tiling modes anywhere (tiling-mode matmuls keep the PE clock
        # gate throttled at 1.2 GHz).
        KT_sb = respool.tile([P, HPC, S], bf16)
        nc.gpsimd.memset(KT_sb[:], 0.0)
        # V with an interleaved ones column per head: head h occupies
        # cols [h*65, h*65+64) and col h*65+64 == 1.0 (softmax denominator
        # rides along the AV matmul as output partition 64).
        V_sb = respool.tile([P, NSO, HPC * (DK + 1)], bf16)
        nc.sync.dma_start(
            V_sb.rearrange("p n (h w) -> p n h w", w=DK + 1)[:, :, :, DK], vones)

        atpool = ctx.enter_context(tc.tile_pool(name="at", bufs=1))
        AT_sb = atpool.tile([P, NPC, S], bf16)   # normalized A^T
        epool = ctx.enter_context(tc.tile_pool(name="expS", bufs=8))
        npool = ctx.enter_context(tc.tile_pool(name="norm", bufs=2))
        opool = ctx.enter_context(tc.tile_pool(name="ostage", bufs=4))

        # ---------------- Phase A: projections ----------------
        with ExitStack() as ctxA:
            wpool = ctxA.enter_context(tc.tile_pool(name="w", bufs=2))
            apool = ctxA.enter_context(tc.tile_pool(name="actT", bufs=2))
            psA = ctxA.enter_context(
                tc.tile_pool(name="psA", bufs=4, space="PSUM"))

            # Q^T and K^T: out[proj-chunk(128), seq(512)] = Wx^T @ xT
            for w_hbm, x_hbm, bias_sb, dst in (
                (wq, qT, bq_sb, QT_sb),
                (wk, kT, bk_sb, KT_sb),
            ):
                w_sb = wpool.tile([P, NDM, PROJ], bf16, tag="w", name="w_sb")
                nc.sync.dma_start(w_sb[:], w_hbm.rearrange("(o p) n -> p o n", p=P))
                for sc in range(NSC):
                    a_sb = apool.tile([P, NDM, 512], bf16, tag="a", name="a_sb")
                    nc.sync.dma_start(
                        a_sb[:],
                        x_hbm.rearrange("(o p) s -> p o s", p=P)[
                            :, :, sc * 512:(sc + 1) * 512],
                    )
                    for pc in range(NPC):
                        ps = psA.tile([P, 512], f32, tag="pp", name="psa")
                        for dc in range(NDM):
                            nc.tensor.matmul(
                                ps,
                                lhsT=r(w_sb[:, dc, pc * P:(pc + 1) * P]),
                                rhs=r(a_sb[:, dc, :]),
                                start=(dc == 0), stop=(dc == NDM - 1),
                            )
                        if dst is QT_sb:
                            nc.vector.tensor_scalar_add(
                                dst[:, pc, sc * 512:(sc + 1) * 512], ps,
                                bias_sb[:, pc:pc + 1])
                        else:
                            for half in range(2):
                                lo = half * 64
                                nc.vector.tensor_scalar_add(
                                    KT_sb[lo:lo + 64, 2 * pc + half,
                                          sc * 512:(sc + 1) * 512],
                                    ps[lo:lo + 64, :],
                                    bias_sb[lo:lo + 64, pc:pc + 1])

            # V: out[seq-chunk(128), proj(512)] = vT^T @ Wv
            w_sb = wpool.tile([P, NDM, PROJ], bf16, tag="w", name="w_sb")
            nc.sync.dma_start(w_sb[:], wv.rearrange("(o p) n -> p o n", p=P))
            for sc in range(NSC):
                a_sb = apool.tile([P, NDM, 512], bf16, tag="a", name="a_sb")
                nc.sync.dma_start(
                    a_sb[:],
                    vT.rearrange("(o p) s -> p o s", p=P)[
                        :, :, sc * 512:(sc + 1) * 512],
                )
                for so4 in range(4):
                    so = sc * 4 + so4
                    ps = psA.tile([P, 512], f32, tag="pp", name="psa")
                    for dc in range(NDM):
                        nc.tensor.matmul(
                            ps,
                            lhsT=r(a_sb[:, dc, so4 * P:(so4 + 1) * P]),
                            rhs=r(w_sb[:, dc, :]),
                            start=(dc == 0), stop=(dc == NDM - 1),
                        )
                    nc.vector.tensor_copy(
                        V_sb[:, so, :].rearrange(
                            "p (h w) -> p h w", w=DK + 1)[:, :, 0:DK],
                        ps.rearrange("p (h w) -> p h w", w=DK))

        # ---------------- Phase B: attention ----------------
        with ExitStack() as ctxB:
            with ExitStack() as ctxBi:
                psS = ctxBi.enter_context(
                    tc.tile_pool(name="psS", bufs=2, space="PSUM"))
                psAcc = ctxBi.enter_context(
                    tc.tile_pool(name="psAcc", bufs=1, space="PSUM"))

                for pr in range(NPC):           # head pairs
                    for qc in range(2):         # query 1024-chunks
                        avs = [
                            psAcc.tile([P, 1024], f32, tag=f"av{hi}",
                                       name=f"av{hi}")
                            for hi in range(2)
                        ]
                        for kc in range(NKC):   # key 128-chunks
                            es = []
                            for hi in range(2):
                                h = 2 * pr + hi
                                sp = psS.tile([P, 1024], f32, tag="s",
                                              name="sp")
                                for sub in range(2):
                                    nc.tensor.matmul(
                                        sp[:, sub * 512:(sub + 1) * 512],
                                        lhsT=r(KT_sb[:, h,
                                                     kc * P:(kc + 1) * P]),
                                        rhs=r(QT_sb[:, pr,
                                                    qc * 1024 + sub * 512:
                                                    qc * 1024 + (sub + 1) * 512]),
                                        start=True, stop=True,
                                    )
                                e = epool.tile([P, 1024], bf16, tag="e",
                                               name="e")
                                nc.scalar.activation(
                                    e, sp, AF.Exp,
                                    bias=mb_sb[:, kc:kc + 1],
                                    scale=float(1.0 / np.sqrt(DK)),
                                )
                                es.append(e)
                            for hi in range(2):
                                h = 2 * pr + hi
                                for sub in range(2):
                                    nc.tensor.matmul(
                                        avs[hi][0:DK + 1,
                                                sub * 512:(sub + 1) * 512],
                                        lhsT=r(V_sb[:, kc,
                                                    h * (DK + 1):
                                                    (h + 1) * (DK + 1)]),
                                        rhs=r(es[hi][:, sub * 512:
                                                     (sub + 1) * 512]),
                                        start=(kc == 0),
                                        stop=(kc == NKC - 1),
                                    )
                        # normalization: A^T *= 1/l (broadcast via selector mm)
                        Lsb4 = npool.tile([P, 1024], f32, tag="lsb",
                                          name="Lsb4")
                        nc.gpsimd.memset(Lsb4[:], 0.0)
                        for hi in range(2):
                            nc.vector.tensor_copy(
                                Lsb4[hi * 32:hi * 32 + 1, :],
                                avs[hi][DK:DK + 1, :])
                        bc = psS.tile([P, 1024], f32, tag="s", name="bc")
                        for sub in range(2):
                            nc.tensor.matmul(
                                bc[:, sub * 512:(sub + 1) * 512],
                                lhsT=r(sel_sb[:]),
                                rhs=r(Lsb4[:, sub * 512:(sub + 1) * 512]),
                                start=True, stop=True,
                            )
                        rc = npool.tile([P, 1024], f32, tag="rc", name="rc")
                        nc.vector.reciprocal(rc, bc)
                        for hi in range(2):
                            nc.vector.tensor_tensor(
                                AT_sb[hi * 64:(hi + 1) * 64, pr,
                                      qc * 1024:(qc + 1) * 1024],
                                avs[hi][0:64, :],
                                rc[hi * 64:(hi + 1) * 64, :], MUL)

            # ---------------- Phase C: output projection ----------------
            with ExitStack() as ctxC:
                psC = ctxC.enter_context(
                    tc.tile_pool(name="psC", bufs=4, space="PSUM"))
                for so in range(NSO):
                    for oc in range(2):
                        ps = psC.tile([P, 512], f32, tag="po", name="pso")
                        for pc in range(NPC):
                            nc.tensor.matmul(
                                ps,
                                lhsT=r(AT_sb[:, pc, so * P:(so + 1) * P]),
                                rhs=r(wo_sb[:, pc, oc * 512:(oc + 1) * 512]),
                                start=(pc == 0), stop=(pc == NPC - 1),
                            )
                        ost = opool.tile([P, 512], f32, tag="o", name="ost")
                        nc.vector.tensor_copy(ost, ps)
                        nc.sync.dma_start(
                            out[so * P:(so + 1) * P, oc * 512:(oc + 1) * 512],
                            ost)

    nc.compile()
    return nc


def _get_nc():
    if "nc" not in _cache:
        _cache["nc"] = _build()
    return _cache["nc"]


def make_in_maps(q, k, v, mask, Wq, bq, Wk, bk, Wv, bv, Wo, bo):
    """Host-side sharding: slice/transpose the full inputs per core."""
    import ml_dtypes
    f = np.float32
    bf = ml_dtypes.bfloat16
    q = np.asarray(q, dtype=f)
    k = np.asarray(k, dtype=f)
    v = np.asarray(v, dtype=f)
    Wq = np.asarray(Wq, dtype=f)
    Wk = np.asarray(Wk, dtype=f)
    Wv = np.asarray(Wv, dtype=f)
    Wo = np.asarray(Wo, dtype=f)
    bq = np.asarray(bq, dtype=f)
    bk = np.asarray(bk, dtype=f)
    mask = np.asarray(mask)

    sel = np.zeros((P, 128), dtype=f)
    sel[0, 0:64] = 1.0
    sel[32, 64:128] = 1.0

    in_maps = []
    for c in range(NCORES):
        b, hg = divmod(c, 2)
        cols = slice(hg * PROJ, (hg + 1) * PROJ)
        mbias = np.where(mask[b, 0, 0, :] == 0, f(MASK_NEG), f(0.0)).astype(f)
        in_maps.append({
            "qT": np.ascontiguousarray(q[b].T).astype(bf),
            "kT": np.ascontiguousarray(k[b].T).astype(bf),
            "vT": np.ascontiguousarray(v[b].T).astype(bf),
            "wq": np.ascontiguousarray(Wq[:, cols]).astype(bf),
            "wk": np.ascontiguousarray(Wk[:, cols]).astype(bf),
            "wv": np.ascontiguousarray(Wv[:, cols]).astype(bf),
            "wo": np.ascontiguousarray(Wo[cols, :]).astype(bf),
            "bq2": np.ascontiguousarray(bq[cols].reshape(NPC, P).T),
            "bk2": np.ascontiguousarray(bk[cols].reshape(NPC, P).T),
            "mb": np.ascontiguousarray(mbias.reshape(NKC, P).T),
            "sel": sel,
            "vones": np.ones((P, NSO, HPC), dtype=bf),
        })
    return in_maps


def combine_outputs(parts, Wv_bv_Wo_bo):
    """Host-side unshard: sum the two head-group partials per batch, add the
    folded bias bv @ Wo + bo."""
    bv, Wo, bo = Wv_bv_Wo_bo
    bo_eff = (np.asarray(bv, np.float32) @ np.asarray(Wo, np.float32)
              + np.asarray(bo, np.float32))
    out = np.empty((B, S, D), dtype=np.float32)
    for b in range(B):
        out[b] = parts[2 * b] + parts[2 * b + 1] + bo_eff
    return out


def _install_axon_ntff_hook():
    """The agent image's antenv lacks axon_hooks; synthesize it and register
    the ctypes NTFF profile hook from trn_boot so trace=True works."""
    import sys
    import types
    if "antenv.axon_hooks" in sys.modules:
        return
    try:
        from trn_agent_boot.trn_boot import _ntff_profile_via_ctypes
        hook = _ntff_profile_via_ctypes("/opt/axon/libaxon_pjrt.so")
    except Exception:
        hook = None
    mod = types.ModuleType("antenv.axon_hooks")
    mod._hook = hook
    mod.get_axon_ntff_profile_hook = lambda: mod._hook
    mod.set_axon_ntff_profile_hook = lambda h: setattr(mod, "_hook", h)
    sys.modules["antenv.axon_hooks"] = mod
    # upload_artifacts wants a fish bucket; keep artifacts local instead.
    import concourse.bass_utils as bu
    bu.upload_artifacts = lambda tmpdir: str(tmpdir)


def kernel(q, k, v, mask, Wq, bq, Wk, bk, Wv, bv, Wo, bo):
    from concourse.bass_utils import run_bass_kernel_spmd

    nc = _get_nc()
    in_maps = make_in_maps(q, k, v, mask, Wq, bq, Wk, bk, Wv, bv, Wo, bo)
    trace = bool(int(os.environ.get("KERNEL_TRACE", "0")))
    if trace:
        try:
            _install_axon_ntff_hook()
        except Exception:
            trace = False
    try:
        res = run_bass_kernel_spmd(
            nc, in_maps, list(range(NCORES)), trace=trace,
            tmpdir=os.environ.get("KERNEL_TRACE_DIR") or None)
    except Exception:
        if not trace:
            raise
        # Trace machinery failed; rerun without it so results still flow.
        res = run_bass_kernel_spmd(nc, in_maps, list(range(NCORES)), trace=False)
    _cache["last_result"] = res
    parts = [res.results[c]["out"] for c in range(NCORES)]
    return combine_outputs(parts, (bv, Wo, bo))



# revision 9
# speedup vs baseline: 1.0774x; 1.0774x over previous
"""Multi-head attention (B=4, S=2048, D=1024, H=16) on 8 Trainium2 cores.

Sharding: core c handles batch b = c//2 and head-group hg = c%2 (8 of the 16
heads, i.e. 512 of the 1024 projection dims).  Every core computes:

    Qc^T = (Wq_cols^T @ q[b]^T)           [512, 2048]   (proj-major layout)
    Kc^T = (Wk_cols^T @ k[b]^T)           [512, 2048]
    Vc   = (v[b] @ Wv_cols) * mask        [2048, 512]   (mask folded into V)
    S^T  = Kc_h @ Qc_h^T per head         (contraction-64, dense pair layout)
    P^T  = exp(S^T/8)                     (ACT engine, no bias operand)
    A^T  = V_h^T @ P^T   and  l = m^T P^T (AV + masked denominator via the
                                           ones/mask column riding the matmul)
    A^T  = A^T * (1/l)                    (off critical path: copy-out, sel
                                           broadcast matmul, fast reciprocal)
    out_partial = A_c @ Wo_rows           [2048, 1024]  (interleaved into the
                                           next query-group's ACT slack)

Host sums the two head-group partials per batch (the "all-reduce after w_o")
and adds the folded bias bv @ Wo + bo.  The attention mask is applied
multiplicatively through V and the denominator column (exp(s)*m == the
reference's additive -inf masking), so the exp activation needs no bias and
nothing mask-related sits on the critical path.

Phase B runs one head at a time, ACT-paced: per key-chunk the emission order
is scores(kc) -> exp(kc) -> AV(kc-1), which keeps the PE FIFO from ever
stalling behind the scalar engine.  The softmax normalization runs entirely
in the shadow of the next head's pipeline.

All matmuls are bf16 with fp32 PSUM accumulation.
"""

import os
import numpy as np

B, S, D = 4, 2048, 1024
H, DK = 16, 64
P = 128
NCORES = 8
HPC = H // 2            # heads per core
PROJ = HPC * DK         # 512 projection dims per core
NDM = D // P            # 8 d_model chunks
NPC = PROJ // P         # 4 head-pair chunks
NSC = S // 512          # 4 seq chunks of 512
NSO = S // P            # 16 seq chunks of 128
NKC = S // P            # 16 key chunks of 128

_cache = {}


def _build():
    """Build + compile the per-core Bass program (same program on all cores)."""
    import concourse.bass as bass
    import concourse.bacc as bacc
    import concourse.mybir as mybir
    import concourse.tile as tile
    from contextlib import ExitStack

    f32 = mybir.dt.float32
    bf16 = mybir.dt.bfloat16
    AF = mybir.ActivationFunctionType
    MUL = mybir.AluOpType.mult

    nc = bacc.Bacc("TRN2", target_bir_lowering=False, debug=False,
                   num_devices=NCORES)

    qT = nc.dram_tensor("qT", [D, S], bf16, kind="ExternalInput").ap()
    kT = nc.dram_tensor("kT", [D, S], bf16, kind="ExternalInput").ap()
    vT = nc.dram_tensor("vT", [D, S], bf16, kind="ExternalInput").ap()
    wq = nc.dram_tensor("wq", [D, PROJ], bf16, kind="ExternalInput").ap()
    wk = nc.dram_tensor("wk", [D, PROJ], bf16, kind="ExternalInput").ap()
    wv = nc.dram_tensor("wv", [D, PROJ], bf16, kind="ExternalInput").ap()
    wo = nc.dram_tensor("wo", [PROJ, D], bf16, kind="ExternalInput").ap()
    bq2 = nc.dram_tensor("bq2", [P, NPC], f32, kind="ExternalInput").ap()
    bk2 = nc.dram_tensor("bk2", [P, NPC], f32, kind="ExternalInput").ap()
    mk = nc.dram_tensor("mk", [P, NSO], f32, kind="ExternalInput").ap()
    sel = nc.dram_tensor("sel", [P, 128], bf16, kind="ExternalInput").ap()
    vones = nc.dram_tensor("vones", [P, NSO, HPC], bf16, kind="ExternalInput").ap()
    out = nc.dram_tensor("out", [S, D], f32, kind="ExternalOutput").ap()

    with tile.TileContext(nc) as tc, ExitStack() as ctx:
        cpool = ctx.enter_context(tc.tile_pool(name="const", bufs=1))
        sel_sb = cpool.tile([P, 128], bf16)
        nc.sync.dma_start(sel_sb[:], sel)
        bq_sb = cpool.tile([P, NPC], f32)
        nc.sync.dma_start(bq_sb[:], bq2)
        bk_sb = cpool.tile([P, NPC], f32)
        nc.sync.dma_start(bk_sb[:], bk2)
        mk_sb = cpool.tile([P, NSO], f32)
        nc.sync.dma_start(mk_sb[:], mk)
        # l values land in rows {0, 32}; all other rows must be finite (the
        # sel matmul's zero rows nullify them) -- one startup memset.
        Lsb = cpool.tile([P, 1024], bf16)
        nc.gpsimd.memset(Lsb[:], 0.0)

        # Long-lived SBUF result tensors.
        respool = ctx.enter_context(tc.tile_pool(name="res", bufs=1))
        # Q^T pair-stacked: rows 0-63 = head 2*pr dims, rows 64-127 = head
        # 2*pr+1 dims.  K^T stored per head on the full 128-partition
        # contraction range: even heads carry data in rows 0-63 (rows 64-127
        # zero), odd heads in rows 64-127 (rows 0-63 zero), so the score
        # matmul is a full-array standard-mode matmul (no PE tiling modes --
        # partial-array matmuls keep the PE clock gate throttled).
        QT_sb = respool.tile([P, NPC, S], bf16)
        KT_sb = respool.tile([P, HPC, S], bf16)
        nc.gpsimd.memset(KT_sb[:], 0.0)
        # V with an interleaved mask column per head: head h occupies
        # cols [h*65, h*65+64) and col h*65+64 == mask (the masked softmax
        # denominator rides the AV matmul as output partition 64).
        V_sb = respool.tile([P, NSO, HPC * (DK + 1)], bf16)
        nc.sync.dma_start(
            V_sb.rearrange("p n (h w) -> p n h w", w=DK + 1)[:, :, :, DK], vones)
        AT_sb = respool.tile([P, NPC, S], bf16)   # normalized A^T

        wopool = ctx.enter_context(tc.tile_pool(name="wo", bufs=1))
        wo_sb = wopool.tile([P, NPC, D], bf16)

        npool = ctx.enter_context(tc.tile_pool(name="norm", bufs=4))
        rcpool = ctx.enter_context(tc.tile_pool(name="rc", bufs=2))
        epool = ctx.enter_context(tc.tile_pool(name="expS", bufs=3))
        opool = ctx.enter_context(tc.tile_pool(name="ostage", bufs=4))

        # ---------------- Phase A: projections ----------------
        with ExitStack() as ctxA:
            wpool = ctxA.enter_context(tc.tile_pool(name="w", bufs=2))
            apool = ctxA.enter_context(tc.tile_pool(name="actT", bufs=2))
            psA = ctxA.enter_context(
                tc.tile_pool(name="psA", bufs=4, space="PSUM"))

            first = True
            for w_hbm, x_hbm, bias_sb, dst in (
                (wq, qT, bq_sb, QT_sb),
                (wk, kT, bk_sb, KT_sb),
            ):
                w_sb = wpool.tile([P, NDM, PROJ], bf16, tag="w", name="w_sb")
                nc.sync.dma_start(w_sb[:], w_hbm.rearrange("(o p) n -> p o n", p=P))
                if first:
                    # wo is only needed in phase C; queue it behind the
                    # first weight so it never delays the first matmul.
                    nc.sync.dma_start(
                        wo_sb[:], wo.rearrange("(o p) n -> p o n", p=P))
                    first = False
                for sc in range(NSC):
                    a_sb = apool.tile([P, NDM, 512], bf16, tag="a", name="a_sb")
                    nc.sync.dma_start(
                        a_sb[:],
                        x_hbm.rearrange("(o p) s -> p o s", p=P)[
                            :, :, sc * 512:(sc + 1) * 512],
                    )
                    for pc in range(NPC):
                        ps = psA.tile([P, 512], f32, tag="pp", name="psa")
                        for dc in range(NDM):
                            nc.tensor.matmul(
                                ps,
                                lhsT=w_sb[:, dc, pc * P:(pc + 1) * P],
                                rhs=a_sb[:, dc, :],
                                start=(dc == 0), stop=(dc == NDM - 1),
                            )
                        if dst is QT_sb:
                            nc.vector.tensor_scalar_add(
                                dst[:, pc, sc * 512:(sc + 1) * 512], ps,
                                bias_sb[:, pc:pc + 1])
                        else:
                            for half in range(2):
                                lo = half * 64
                                nc.vector.tensor_scalar_add(
                                    KT_sb[lo:lo + 64, 2 * pc + half,
                                          sc * 512:(sc + 1) * 512],
                                    ps[lo:lo + 64, :],
                                    bias_sb[lo:lo + 64, pc:pc + 1])

            # V: out[seq-chunk(128), proj(512)] = vT^T @ Wv, scaled by mask
            w_sb = wpool.tile([P, NDM, PROJ], bf16, tag="w", name="w_sb")
            nc.sync.dma_start(w_sb[:], wv.rearrange("(o p) n -> p o n", p=P))
            for sc in range(NSC):
                a_sb = apool.tile([P, NDM, 512], bf16, tag="a", name="a_sb")
                nc.sync.dma_start(
                    a_sb[:],
                    vT.rearrange("(o p) s -> p o s", p=P)[
                        :, :, sc * 512:(sc + 1) * 512],
                )
                for so4 in range(4):
                    so = sc * 4 + so4
                    ps = psA.tile([P, 512], f32, tag="pp", name="psa")
                    for dc in range(NDM):
                        nc.tensor.matmul(
                            ps,
                            lhsT=a_sb[:, dc, so4 * P:(so4 + 1) * P],
                            rhs=w_sb[:, dc, :],
                            start=(dc == 0), stop=(dc == NDM - 1),
                        )
                    nc.vector.tensor_scalar_mul(
                        V_sb[:, so, :].rearrange(
                            "p (h w) -> p h w", w=DK + 1)[:, :, 0:DK],
                        ps.rearrange("p (h w) -> p h w", w=DK),
                        mk_sb[:, so:so + 1])

        # ---------------- Phases B + C, software pipelined ----------------
        with ExitStack() as ctxB:
            psS = ctxB.enter_context(
                tc.tile_pool(name="psS", bufs=2, space="PSUM"))
            psAcc = ctxB.enter_context(
                tc.tile_pool(name="psAcc", bufs=1, space="PSUM"))
            psX = ctxB.enter_context(
                tc.tile_pool(name="psX", bufs=2, space="PSUM"))

            def emit_c_chunk(qc, j):
                """Output projection for chunk j of query group qc."""
                so = qc * 8 + j // 2
                oc = j % 2
                ps = psX.tile([P, 512], f32, tag="px", name="psx")
                for pc in range(NPC):
                    nc.tensor.matmul(
                        ps,
                        lhsT=AT_sb[:, pc, so * P:(so + 1) * P],
                        rhs=wo_sb[:, pc, oc * 512:(oc + 1) * 512],
                        start=(pc == 0), stop=(pc == NPC - 1),
                    )
                ost = opool.tile([P, 512], f32, tag="o", name="ost")
                nc.vector.tensor_copy(ost, ps)
                nc.sync.dma_start(
                    out[so * P:(so + 1) * P, oc * 512:(oc + 1) * 512], ost)

            def pair_tail(pr, qc, a0, a1):
                """Broadcast 1/l and normalize both heads' A^T (background)."""
                bc = psS.tile([P, 1024], f32, tag="s", name="sp")
                for sub in range(2):
                    nc.tensor.matmul(
                        bc[:, sub * 512:(sub + 1) * 512],
                        lhsT=sel_sb[:],
                        rhs=Lsb[:, sub * 512:(sub + 1) * 512],
                        start=True, stop=True,
                    )
                # Stage bc out of PSUM first (frees the borrowed score
                # buffer in ~1.2us), then run the slow iterative reciprocal
                # entirely in SBUF where it blocks nothing.
                bcs = rcpool.tile([P, 1024], f32, tag="bcs", name="bcs")
                nc.vector.tensor_copy(bcs, bc)
                rc = rcpool.tile([P, 1024], f32, tag="rc", name="rc")
                nc.vector.reciprocal(rc, bcs)
                for hi, at in ((0, a0), (1, a1)):
                    rows = slice(hi * 64, (hi + 1) * 64)
                    nc.vector.tensor_tensor(
                        AT_sb[rows, pr, qc * 1024:(qc + 1) * 1024],
                        at[rows, :], rc[rows, :], MUL)

            atn = [None, None]
            pending = []        # deferred PE/DVE work, emitted inside blocks
            cqueue = []         # deferred phase-C chunks

            for qc in range(2):
                for pr in range(NPC):
                    for hi in range(2):
                        h = 2 * pr + hi
                        avs = psAcc.tile([P, 1024], f32, tag="av", name="avs")
                        es = [None] * NKC
                        for kc in range(NKC):
                            sp = psS.tile([P, 1024], f32, tag="s", name="sp")
                            for sub in range(2):
                                nc.tensor.matmul(
                                    sp[:, sub * 512:(sub + 1) * 512],
                                    lhsT=KT_sb[:, h, kc * P:(kc + 1) * P],
                                    rhs=QT_sb[:, pr,
                                              qc * 1024 + sub * 512:
                                              qc * 1024 + (sub + 1) * 512],
                                    start=True, stop=True,
                                )
                            e = epool.tile([P, 1024], bf16, tag="e", name="e")
                            nc.scalar.activation(
                                e, sp, AF.Exp, scale=float(1.0 / np.sqrt(DK)))
                            es[kc] = e
                            if kc == 2 and pending:
                                # previous pair's normalization: bc matmul +
                                # fast reciprocal + scale, all off critical path
                                pending.pop(0)()
                            if kc in (5, 9, 13) and cqueue:
                                cqueue.pop(0)()
                            if kc > 0:
                                for sub in range(2):
                                    nc.tensor.matmul(
                                        avs[0:DK + 1, sub * 512:(sub + 1) * 512],
                                        lhsT=V_sb[:, kc - 1,
                                                  h * (DK + 1):(h + 1) * (DK + 1)],
                                        rhs=es[kc - 1][:, sub * 512:(sub + 1) * 512],
                                        start=(kc - 1 == 0), stop=False,
                                    )
                        for sub in range(2):
                            nc.tensor.matmul(
                                avs[0:DK + 1, sub * 512:(sub + 1) * 512],
                                lhsT=V_sb[:, NKC - 1,
                                          h * (DK + 1):(h + 1) * (DK + 1)],
                                rhs=es[NKC - 1][:, sub * 512:(sub + 1) * 512],
                                start=False, stop=True,
                            )
                        # Tail: move l and unnormalized A^T out of PSUM so the
                        # next head's AV accumulation can claim the bank.
                        nc.vector.tensor_copy(
                            Lsb[hi * 32:hi * 32 + 1, :], avs[DK:DK + 1, :])
                        # Stage A^T into the partition range matching this
                        # head's rc rows (all-SBUF DVE ops need identical
                        # partition windows; PSUM sources are exempt).
                        at = npool.tile([P, 1024], f32, tag="at", name="at")
                        nc.vector.tensor_copy(
                            at[hi * 64:(hi + 1) * 64, :], avs[0:64, :])
                        atn[hi] = at
                    pending.append(
                        (lambda pr=pr, qc=qc, a0=atn[0], a1=atn[1]:
                         pair_tail(pr, qc, a0, a1)))
                    if qc == 1 and pr == 0:
                        # queue the previous query group's output projection
                        cqueue.extend(
                            [(lambda j=j: emit_c_chunk(0, j))
                             for j in range(16)])

            # drain: last pair's normalization, then the last output chunks
            while pending:
                pending.pop(0)()
            for j in range(16):
                emit_c_chunk(1, j)
            while cqueue:       # any phase-C chunks not consumed in-loop
                cqueue.pop(0)()

    nc.compile()
    return nc


def _get_nc():
    if "nc" not in _cache:
        _cache["nc"] = _build()
    return _cache["nc"]


def make_in_maps(q, k, v, mask, Wq, bq, Wk, bk, Wv, bv, Wo, bo):
    """Host-side sharding: slice/transpose the full inputs per core."""
    import ml_dtypes
    f = np.float32
    bf = ml_dtypes.bfloat16
    q = np.asarray(q, dtype=f)
    k = np.asarray(k, dtype=f)
    v = np.asarray(v, dtype=f)
    Wq = np.asarray(Wq, dtype=f)
    Wk = np.asarray(Wk, dtype=f)
    Wv = np.asarray(Wv, dtype=f)
    Wo = np.asarray(Wo, dtype=f)
    bq = np.asarray(bq, dtype=f)
    bk = np.asarray(bk, dtype=f)
    mask = np.asarray(mask)

    sel = np.zeros((P, 128), dtype=f)
    sel[0, 0:64] = 1.0
    sel[32, 64:128] = 1.0

    in_maps = []
    for c in range(NCORES):
        b, hg = divmod(c, 2)
        cols = slice(hg * PROJ, (hg + 1) * PROJ)
        mvals = (mask[b, 0, 0, :] != 0).astype(f)            # [S]
        mk2 = np.ascontiguousarray(mvals.reshape(NSO, P).T)  # [P, NSO]
        in_maps.append({
            "qT": np.ascontiguousarray(q[b].T).astype(bf),
            "kT": np.ascontiguousarray(k[b].T).astype(bf),
            "vT": np.ascontiguousarray(v[b].T).astype(bf),
            "wq": np.ascontiguousarray(Wq[:, cols]).astype(bf),
            "wk": np.ascontiguousarray(Wk[:, cols]).astype(bf),
            "wv": np.ascontiguousarray(Wv[:, cols]).astype(bf),
            "wo": np.ascontiguousarray(Wo[cols, :]).astype(bf),
            "bq2": np.ascontiguousarray(bq[cols].reshape(NPC, P).T),
            "bk2": np.ascontiguousarray(bk[cols].reshape(NPC, P).T),
            "mk": mk2,
            "sel": sel.astype(bf),
            "vones": np.broadcast_to(
                mk2.astype(bf)[:, :, None], (P, NSO, HPC)).copy(),
        })
    return in_maps


def combine_outputs(parts, Wv_bv_Wo_bo):
    """Host-side unshard: sum the two head-group partials per batch, add the
    folded bias bv @ Wo + bo."""
    bv, Wo, bo = Wv_bv_Wo_bo
    bo_eff = (np.asarray(bv, np.float32) @ np.asarray(Wo, np.float32)
              + np.asarray(bo, np.float32))
    out = np.empty((B, S, D), dtype=np.float32)
    for b in range(B):
        out[b] = parts[2 * b] + parts[2 * b + 1] + bo_eff
    return out


def _install_axon_ntff_hook():
    """The agent image's antenv lacks axon_hooks; synthesize it and register
    the ctypes NTFF profile hook from trn_boot so trace=True works."""
    import sys
    import types
    if "antenv.axon_hooks" in sys.modules:
        return
    try:
        from trn_agent_boot.trn_boot import _ntff_profile_via_ctypes
        hook = _ntff_profile_via_ctypes("/opt/axon/libaxon_pjrt.so")
    except Exception:
        hook = None
    mod = types.ModuleType("antenv.axon_hooks")
    mod._hook = hook
    mod.get_axon_ntff_profile_hook = lambda: mod._hook
    mod.set_axon_ntff_profile_hook = lambda h: setattr(mod, "_hook", h)
    sys.modules["antenv.axon_hooks"] = mod
    # upload_artifacts wants a fish bucket; keep artifacts local instead.
    import concourse.bass_utils as bu
    bu.upload_artifacts = lambda tmpdir: str(tmpdir)


def kernel(q, k, v, mask, Wq, bq, Wk, bk, Wv, bv, Wo, bo):
    from concourse.bass_utils import run_bass_kernel_spmd

    nc = _get_nc()
    in_maps = make_in_maps(q, k, v, mask, Wq, bq, Wk, bk, Wv, bv, Wo, bo)
    trace = bool(int(os.environ.get("KERNEL_TRACE", "0")))
    if trace:
        try:
            _install_axon_ntff_hook()
        except Exception:
            trace = False
    try:
        res = run_bass_kernel_spmd(
            nc, in_maps, list(range(NCORES)), trace=trace,
            tmpdir=os.environ.get("KERNEL_TRACE_DIR") or None)
    except Exception:
        if not trace:
            raise
        # Trace machinery failed; rerun without it so results still flow.
        res = run_bass_kernel_spmd(nc, in_maps, list(range(NCORES)), trace=False)
    _cache["last_result"] = res
    parts = [res.results[c]["out"] for c in range(NCORES)]
    return combine_outputs(parts, (bv, Wo, bo))


# revision 13
# speedup vs baseline: 1.1413x; 1.0593x over previous
"""Multi-head attention (B=4, S=2048, D=1024, H=16) on 8 Trainium2 cores.

Sharding: core c handles batch b = c//2 and head-group hg = c%2 (8 of the 16
heads, i.e. 512 of the 1024 projection dims).  Every core computes:

    Qc^T = (Wq_cols^T @ q[b]^T)           [512, 2048]   (proj-major layout)
    Kc^T = (Wk_cols^T @ k[b]^T)           [512, 2048]
    Vc   = (v[b] @ Wv_cols) * mask        [2048, 512]   (mask folded into V)
    S^T  = Kc_h @ Qc_h^T per head         (contraction-64, dense pair layout)
    P^T  = exp(S^T/8)                     (ACT engine, no bias operand)
    A^T  = V_h^T @ P^T   and  l = m^T P^T (AV + masked denominator via the
                                           ones/mask column riding the matmul)
    A^T  = A^T * (1/l)                    (off critical path: copy-out, sel
                                           broadcast matmul, fast reciprocal)
    out_partial = A_c @ Wo_rows           [2048, 1024]  (interleaved into the
                                           next query-group's ACT slack)

Host sums the two head-group partials per batch (the "all-reduce after w_o")
and adds the folded bias bv @ Wo + bo.  The attention mask is applied
multiplicatively through V and the denominator column (exp(s)*m == the
reference's additive -inf masking), so the exp activation needs no bias and
nothing mask-related sits on the critical path.

Phase B runs one head at a time, ACT-paced: per key-chunk the emission order
is scores(kc) -> exp(kc) -> AV(kc-1), which keeps the PE FIFO from ever
stalling behind the scalar engine.  The softmax normalization runs entirely
in the shadow of the next head's pipeline.

All matmuls are bf16 with fp32 PSUM accumulation.
"""

import os
import numpy as np

B, S, D = 4, 2048, 1024
H, DK = 16, 64
P = 128
NCORES = 8
HPC = H // 2            # heads per core
PROJ = HPC * DK         # 512 projection dims per core
NDM = D // P            # 8 d_model chunks
NPC = PROJ // P         # 4 head-pair chunks
NSC = S // 512          # 4 seq chunks of 512
NSO = S // P            # 16 seq chunks of 128
NKC = S // P            # 16 key chunks of 128

_cache = {}


def _build():
    """Build + compile the per-core Bass program (same program on all cores)."""
    import concourse.bass as bass
    import concourse.bacc as bacc
    import concourse.mybir as mybir
    import concourse.tile as tile
    from contextlib import ExitStack

    f32 = mybir.dt.float32
    bf16 = mybir.dt.bfloat16
    AF = mybir.ActivationFunctionType
    MUL = mybir.AluOpType.mult

    nc = bacc.Bacc("TRN2", target_bir_lowering=False, debug=False,
                   num_devices=NCORES)

    qT = nc.dram_tensor("qT", [D, S], bf16, kind="ExternalInput").ap()
    kT = nc.dram_tensor("kT", [D, S], bf16, kind="ExternalInput").ap()
    vT = nc.dram_tensor("vT", [D, S], bf16, kind="ExternalInput").ap()
    wq = nc.dram_tensor("wq", [D, PROJ], bf16, kind="ExternalInput").ap()
    wk = nc.dram_tensor("wk", [D, PROJ], bf16, kind="ExternalInput").ap()
    wv = nc.dram_tensor("wv", [D, PROJ], bf16, kind="ExternalInput").ap()
    wo = nc.dram_tensor("wo", [PROJ, D], bf16, kind="ExternalInput").ap()
    bq2 = nc.dram_tensor("bq2", [P, NPC], f32, kind="ExternalInput").ap()
    bk2 = nc.dram_tensor("bk2", [P, NPC], f32, kind="ExternalInput").ap()
    mk = nc.dram_tensor("mk", [P, NSO], f32, kind="ExternalInput").ap()
    sel = nc.dram_tensor("sel", [P, 128], bf16, kind="ExternalInput").ap()
    vones = nc.dram_tensor("vones", [P, NSO, HPC], bf16, kind="ExternalInput").ap()
    out = nc.dram_tensor("out", [S, D], f32, kind="ExternalOutput").ap()

    with tile.TileContext(nc) as tc, ExitStack() as ctx:
        cpool = ctx.enter_context(tc.tile_pool(name="const", bufs=1))
        sel_sb = cpool.tile([P, 128], bf16)
        nc.sync.dma_start(sel_sb[:], sel)
        bq_sb = cpool.tile([P, NPC], f32)
        nc.sync.dma_start(bq_sb[:], bq2)
        bk_sb = cpool.tile([P, NPC], f32)
        nc.sync.dma_start(bk_sb[:], bk2)
        mk_sb = cpool.tile([P, NSO], f32)
        nc.sync.dma_start(mk_sb[:], mk)
        # l values land in rows {0, 32}; all other rows must be finite (the
        # sel matmul's zero rows nullify them) -- one startup memset.
        Lsb = cpool.tile([P, 1024], bf16)
        nc.gpsimd.memset(Lsb[:], 0.0)

        # Long-lived SBUF result tensors.
        respool = ctx.enter_context(tc.tile_pool(name="res", bufs=1))
        # Q^T pair-stacked: rows 0-63 = head 2*pr dims, rows 64-127 = head
        # 2*pr+1 dims.  K^T stored per head on the full 128-partition
        # contraction range: even heads carry data in rows 0-63 (rows 64-127
        # zero), odd heads in rows 64-127 (rows 0-63 zero), so the score
        # matmul is a full-array standard-mode matmul (no PE tiling modes --
        # partial-array matmuls keep the PE clock gate throttled).
        QT_sb = respool.tile([P, NPC, S], bf16)
        KT_sb = respool.tile([P, HPC, S], bf16)
        nc.gpsimd.memset(KT_sb[:], 0.0)
        # V with an interleaved mask column per head: head h occupies
        # cols [h*65, h*65+64) and col h*65+64 == mask (the masked softmax
        # denominator rides the AV matmul as output partition 64).
        # (Its DMA is queued later so it can't delay the first projections.)
        V_sb = respool.tile([P, NSO, HPC * (DK + 1)], bf16)
        AT_sb = respool.tile([P, NPC, S], bf16)   # normalized A^T

        wopool = ctx.enter_context(tc.tile_pool(name="wo", bufs=1))
        wo_sb = wopool.tile([P, NPC, D], bf16)

        npool = ctx.enter_context(tc.tile_pool(name="norm", bufs=4))
        rcpool = ctx.enter_context(tc.tile_pool(name="rc", bufs=2))
        epool = ctx.enter_context(tc.tile_pool(name="expS", bufs=3))
        opool = ctx.enter_context(tc.tile_pool(name="ostage", bufs=4))

        # ---------------- Phase A: projections ----------------
        with ExitStack() as ctxA:
            wpool = ctxA.enter_context(tc.tile_pool(name="w", bufs=2))
            apool = ctxA.enter_context(tc.tile_pool(name="actT", bufs=2))
            psA = ctxA.enter_context(
                tc.tile_pool(name="psA", bufs=4, space="PSUM"))

            first = True
            for w_hbm, x_hbm, bias_sb, dst in (
                (wq, qT, bq_sb, QT_sb),
                (wk, kT, bk_sb, KT_sb),
            ):
                w_sb = wpool.tile([P, NDM, PROJ], bf16, tag="w", name="w_sb")
                nc.sync.dma_start(w_sb[:], w_hbm.rearrange("(o p) n -> p o n", p=P))
                if not first:
                    # vones/wo are needed only by phase B's AV loop and
                    # phase C; queue them behind the K weights so they never
                    # delay the first projection matmuls.
                    nc.sync.dma_start(
                        V_sb.rearrange("p n (h w) -> p n h w", w=DK + 1)
                        [:, :, :, DK], vones)
                    nc.sync.dma_start(
                        wo_sb[:], wo.rearrange("(o p) n -> p o n", p=P))
                first = False
                for sc in range(NSC):
                    a_sb = apool.tile([P, NDM, 512], bf16, tag="a", name="a_sb")
                    nc.sync.dma_start(
                        a_sb[:],
                        x_hbm.rearrange("(o p) s -> p o s", p=P)[
                            :, :, sc * 512:(sc + 1) * 512],
                    )
                    for pc in range(NPC):
                        ps = psA.tile([P, 512], f32, tag="pp", name="psa")
                        for dc in range(NDM):
                            nc.tensor.matmul(
                                ps,
                                lhsT=w_sb[:, dc, pc * P:(pc + 1) * P],
                                rhs=a_sb[:, dc, :],
                                start=(dc == 0), stop=(dc == NDM - 1),
                            )
                        if dst is QT_sb:
                            nc.vector.tensor_scalar_add(
                                dst[:, pc, sc * 512:(sc + 1) * 512], ps,
                                bias_sb[:, pc:pc + 1])
                        else:
                            for half in range(2):
                                lo = half * 64
                                nc.vector.tensor_scalar_add(
                                    KT_sb[lo:lo + 64, 2 * pc + half,
                                          sc * 512:(sc + 1) * 512],
                                    ps[lo:lo + 64, :],
                                    bias_sb[lo:lo + 64, pc:pc + 1])

            # V: out[seq-chunk(128), proj(512)] = vT^T @ Wv, scaled by mask
            w_sb = wpool.tile([P, NDM, PROJ], bf16, tag="w", name="w_sb")
            nc.sync.dma_start(w_sb[:], wv.rearrange("(o p) n -> p o n", p=P))
            for sc in range(NSC):
                a_sb = apool.tile([P, NDM, 512], bf16, tag="a", name="a_sb")
                nc.sync.dma_start(
                    a_sb[:],
                    vT.rearrange("(o p) s -> p o s", p=P)[
                        :, :, sc * 512:(sc + 1) * 512],
                )
                for so4 in range(4):
                    so = sc * 4 + so4
                    ps = psA.tile([P, 512], f32, tag="pp", name="psa")
                    for dc in range(NDM):
                        nc.tensor.matmul(
                            ps,
                            lhsT=a_sb[:, dc, so4 * P:(so4 + 1) * P],
                            rhs=w_sb[:, dc, :],
                            start=(dc == 0), stop=(dc == NDM - 1),
                        )
                    nc.vector.tensor_scalar_mul(
                        V_sb[:, so, :].rearrange(
                            "p (h w) -> p h w", w=DK + 1)[:, :, 0:DK],
                        ps.rearrange("p (h w) -> p h w", w=DK),
                        mk_sb[:, so:so + 1])

        # ---------------- Phases B + C, software pipelined ----------------
        with ExitStack() as ctxB:
            psS = ctxB.enter_context(
                tc.tile_pool(name="psS", bufs=2, space="PSUM"))
            psAcc = ctxB.enter_context(
                tc.tile_pool(name="psAcc", bufs=1, space="PSUM"))
            psX = ctxB.enter_context(
                tc.tile_pool(name="psX", bufs=2, space="PSUM"))

            def emit_c_chunk(qc, j):
                """Output projection for chunk j of query group qc."""
                so = qc * 8 + j // 2
                oc = j % 2
                ps = psX.tile([P, 512], f32, tag="px", name="psx")
                for pc in range(NPC):
                    nc.tensor.matmul(
                        ps,
                        lhsT=AT_sb[:, pc, so * P:(so + 1) * P],
                        rhs=wo_sb[:, pc, oc * 512:(oc + 1) * 512],
                        start=(pc == 0), stop=(pc == NPC - 1),
                    )
                ost = opool.tile([P, 512], f32, tag="o", name="ost")
                nc.vector.tensor_copy(ost, ps)
                nc.sync.dma_start(
                    out[so * P:(so + 1) * P, oc * 512:(oc + 1) * 512], ost)

            def pair_tail(pr, qc, a0, a1):
                """Broadcast 1/l and normalize both heads' A^T (background)."""
                bc = psS.tile([P, 1024], f32, tag="s", name="sp")
                for sub in range(2):
                    nc.tensor.matmul(
                        bc[:, sub * 512:(sub + 1) * 512],
                        lhsT=sel_sb[:],
                        rhs=Lsb[:, sub * 512:(sub + 1) * 512],
                        start=True, stop=True,
                    )
                # Stage bc out of PSUM first (frees the borrowed score
                # buffer in ~1.2us), then run the slow iterative reciprocal
                # entirely in SBUF where it blocks nothing.
                bcs = rcpool.tile([P, 1024], f32, tag="bcs", name="bcs")
                nc.vector.tensor_copy(bcs, bc)
                rc = rcpool.tile([P, 1024], f32, tag="rc", name="rc")
                nc.vector.reciprocal(rc, bcs)
                for hi, at in ((0, a0), (1, a1)):
                    rows = slice(hi * 64, (hi + 1) * 64)
                    nc.vector.tensor_tensor(
                        AT_sb[rows, pr, qc * 1024:(qc + 1) * 1024],
                        at[rows, :], rc[rows, :], MUL)

            atn = [None, None]
            pending = []        # deferred PE/DVE work, emitted inside blocks
            cqueue = []         # deferred phase-C chunks
            carry = None        # (h, avs, es, hi) AV(14..15)+tail of prev block

            def emit_av(h, avs, es, kc, start, stop):
                for sub in range(2):
                    nc.tensor.matmul(
                        avs[0:DK + 1, sub * 512:(sub + 1) * 512],
                        lhsT=V_sb[:, kc, h * (DK + 1):(h + 1) * (DK + 1)],
                        rhs=es[kc][:, sub * 512:(sub + 1) * 512],
                        start=start, stop=stop,
                    )

            def emit_block_tail(h, avs, es, hi):
                """Last AV + the PSUM evacuation copies for a block."""
                emit_av(h, avs, es, NKC - 1, False, True)
                nc.vector.tensor_copy(
                    Lsb[hi * 32:hi * 32 + 1, :], avs[DK:DK + 1, :])
                # Stage A^T into the partition range matching this head's rc
                # rows (all-SBUF DVE ops need identical partition windows;
                # PSUM sources are exempt).
                at = npool.tile([P, 1024], f32, tag="at", name="at")
                nc.vector.tensor_copy(
                    at[hi * 64:(hi + 1) * 64, :], avs[0:64, :])
                atn[hi] = at

            blocks = [(qc, pr, hi)
                      for qc in range(2) for pr in range(NPC) for hi in range(2)]
            for qc, pr, hi in blocks:
                h = 2 * pr + hi
                es = [None] * NKC
                avs = None
                for kc in range(NKC):
                    sp = psS.tile([P, 1024], f32, tag="s", name="sp")
                    for sub in range(2):
                        nc.tensor.matmul(
                            sp[:, sub * 512:(sub + 1) * 512],
                            lhsT=KT_sb[:, h, kc * P:(kc + 1) * P],
                            rhs=QT_sb[:, pr,
                                      qc * 1024 + sub * 512:
                                      qc * 1024 + (sub + 1) * 512],
                            start=True, stop=True,
                        )
                    e = epool.tile([P, 1024], bf16, tag="e", name="e")
                    nc.scalar.activation(
                        e, sp, AF.Exp, scale=float(1.0 / np.sqrt(DK)))
                    es[kc] = e
                    if kc == 1 and carry is not None:
                        # previous block's last AVs + PSUM evacuation run
                        # under this block's first scores/exps
                        emit_block_tail(*carry)
                        carry = None
                        if hi == 0:
                            pending.append(
                                (lambda pr2=pr - 1 if pr else NPC - 1,
                                 qc2=qc if pr else qc - 1,
                                 a0=atn[0], a1=atn[1]:
                                 pair_tail(pr2, qc2, a0, a1)))
                    if kc == 2:
                        # this block's AV accumulator (allocated after the
                        # previous block's evacuation copies for clean WAR)
                        avs = psAcc.tile([P, 1024], f32, tag="av", name="avs")
                        emit_av(h, avs, es, 0, True, False)
                        emit_av(h, avs, es, 1, False, False)
                    if kc == 3 and pending:
                        # previous pair's normalization: bc matmul +
                        # reciprocal + scale, all off the critical path
                        pending.pop(0)()
                    if kc in (6, 10, 13) and cqueue:
                        tc.no_sync_barrier()   # pin C chunks at this point
                        cqueue.pop(0)()
                    if kc > 2:
                        emit_av(h, avs, es, kc - 1, False, False)
                if (qc, pr, hi) == (1, 0, 1):
                    # queue the first query group's output projection; it is
                    # consumed (fenced) inside the remaining qc=1 blocks
                    cqueue.extend(
                        [(lambda j=j: emit_c_chunk(0, j)) for j in range(16)])
                carry = (h, avs, es, hi)

            # drain: last block tail, last pair's normalization, final outputs
            emit_block_tail(*carry)
            pending.append(
                (lambda a0=atn[0], a1=atn[1]:
                 pair_tail(NPC - 1, 1, a0, a1)))
            while pending:
                pending.pop(0)()
            while cqueue:       # any phase-C chunks not consumed in-loop
                tc.no_sync_barrier()
                cqueue.pop(0)()
            for j in range(16):
                emit_c_chunk(1, j)

    nc.compile()
    return nc


def _get_nc():
    if "nc" not in _cache:
        _cache["nc"] = _build()
    return _cache["nc"]


def make_in_maps(q, k, v, mask, Wq, bq, Wk, bk, Wv, bv, Wo, bo):
    """Host-side sharding: slice/transpose the full inputs per core."""
    import ml_dtypes
    f = np.float32
    bf = ml_dtypes.bfloat16
    q = np.asarray(q, dtype=f)
    k = np.asarray(k, dtype=f)
    v = np.asarray(v, dtype=f)
    Wq = np.asarray(Wq, dtype=f)
    Wk = np.asarray(Wk, dtype=f)
    Wv = np.asarray(Wv, dtype=f)
    Wo = np.asarray(Wo, dtype=f)
    bq = np.asarray(bq, dtype=f)
    bk = np.asarray(bk, dtype=f)
    mask = np.asarray(mask)

    sel = np.zeros((P, 128), dtype=f)
    sel[0, 0:64] = 1.0
    sel[32, 64:128] = 1.0

    in_maps = []
    for c in range(NCORES):
        b, hg = divmod(c, 2)
        cols = slice(hg * PROJ, (hg + 1) * PROJ)
        mvals = (mask[b, 0, 0, :] != 0).astype(f)            # [S]
        mk2 = np.ascontiguousarray(mvals.reshape(NSO, P).T)  # [P, NSO]
        in_maps.append({
            "qT": np.ascontiguousarray(q[b].T).astype(bf),
            "kT": np.ascontiguousarray(k[b].T).astype(bf),
            "vT": np.ascontiguousarray(v[b].T).astype(bf),
            "wq": np.ascontiguousarray(Wq[:, cols]).astype(bf),
            "wk": np.ascontiguousarray(Wk[:, cols]).astype(bf),
            "wv": np.ascontiguousarray(Wv[:, cols]).astype(bf),
            "wo": np.ascontiguousarray(Wo[cols, :]).astype(bf),
            "bq2": np.ascontiguousarray(bq[cols].reshape(NPC, P).T),
            "bk2": np.ascontiguousarray(bk[cols].reshape(NPC, P).T),
            "mk": mk2,
            "sel": sel.astype(bf),
            "vones": np.broadcast_to(
                mk2.astype(bf)[:, :, None], (P, NSO, HPC)).copy(),
        })
    return in_maps


def combine_outputs(parts, Wv_bv_Wo_bo):
    """Host-side unshard: sum the two head-group partials per batch, add the
    folded bias bv @ Wo + bo."""
    bv, Wo, bo = Wv_bv_Wo_bo
    bo_eff = (np.asarray(bv, np.float32) @ np.asarray(Wo, np.float32)
              + np.asarray(bo, np.float32))
    out = np.empty((B, S, D), dtype=np.float32)
    for b in range(B):
        out[b] = parts[2 * b] + parts[2 * b + 1] + bo_eff
    return out


def _install_axon_ntff_hook():
    """The agent image's antenv lacks axon_hooks; synthesize it and register
    the ctypes NTFF profile hook from trn_boot so trace=True works."""
    import sys
    import types
    if "antenv.axon_hooks" in sys.modules:
        return
    try:
        from trn_agent_boot.trn_boot import _ntff_profile_via_ctypes
        hook = _ntff_profile_via_ctypes("/opt/axon/libaxon_pjrt.so")
    except Exception:
        hook = None
    mod = types.ModuleType("antenv.axon_hooks")
    mod._hook = hook
    mod.get_axon_ntff_profile_hook = lambda: mod._hook
    mod.set_axon_ntff_profile_hook = lambda h: setattr(mod, "_hook", h)
    sys.modules["antenv.axon_hooks"] = mod
    # upload_artifacts wants a fish bucket; keep artifacts local instead.
    import concourse.bass_utils as bu
    bu.upload_artifacts = lambda tmpdir: str(tmpdir)


def kernel(q, k, v, mask, Wq, bq, Wk, bk, Wv, bv, Wo, bo):
    from concourse.bass_utils import run_bass_kernel_spmd

    nc = _get_nc()
    in_maps = make_in_maps(q, k, v, mask, Wq, bq, Wk, bk, Wv, bv, Wo, bo)
    trace = bool(int(os.environ.get("KERNEL_TRACE", "0")))
    if trace:
        try:
            _install_axon_ntff_hook()
        except Exception:
            trace = False
    try:
        res = run_bass_kernel_spmd(
            nc, in_maps, list(range(NCORES)), trace=trace,
            tmpdir=os.environ.get("KERNEL_TRACE_DIR") or None)
    except Exception:
        if not trace:
            raise
        # Trace machinery failed; rerun without it so results still flow.
        res = run_bass_kernel_spmd(nc, in_maps, list(range(NCORES)), trace=False)
    _cache["last_result"] = res
    parts = [res.results[c]["out"] for c in range(NCORES)]
    return combine_outputs(parts, (bv, Wo, bo))


# revision 25
# speedup vs baseline: 1.2508x; 1.0959x over previous
"""Multi-head attention (B=4, S=2048, D=1024, H=16) on 8 Trainium2 cores.

Sharding: core c handles batch b = c//2 and head-group hg = c%2 (8 of the 16
heads, i.e. 512 of the 1024 projection dims).  Every core computes:

    Qc^T = (Wq_cols^T @ q[b]^T)           [512, 2048]   (proj-major layout)
    Kc^T = (Wk_cols^T @ k[b]^T)           [512, 2048]
    Vc   = (v[b] @ Wv_cols) * mask        [2048, 512]   (mask folded into V)
    S^T  = Kc_h @ Qc_h^T per head         (contraction-64, dense pair layout)
    P^T  = exp(S^T/8)                     (ACT engine, no bias operand)
    A^T  = V_h^T @ P^T   and  l = m^T P^T (AV + masked denominator via the
                                           ones/mask column riding the matmul)
    A^T  = A^T * (1/l)                    (off critical path: copy-out, sel
                                           broadcast matmul, fast reciprocal)
    out_partial = A_c @ Wo_rows           [2048, 1024]  (interleaved into the
                                           next query-group's ACT slack)

Host sums the two head-group partials per batch (the "all-reduce after w_o")
and adds the folded bias bv @ Wo + bo.  The attention mask is applied
multiplicatively through V and the denominator column (exp(s)*m == the
reference's additive -inf masking), so the exp activation needs no bias and
nothing mask-related sits on the critical path.

Phase B runs one head at a time, ACT-paced: per key-chunk the emission order
is scores(kc) -> exp(kc) -> AV(kc-1), which keeps the PE FIFO from ever
stalling behind the scalar engine.  The softmax normalization runs entirely
in the shadow of the next head's pipeline.

All matmuls are bf16 with fp32 PSUM accumulation.
"""

import os
import numpy as np

B, S, D = 4, 2048, 1024
H, DK = 16, 64
P = 128
NCORES = 8
HPC = H // 2            # heads per core
PROJ = HPC * DK         # 512 projection dims per core
NDM = D // P            # 8 d_model chunks
NPC = PROJ // P         # 4 head-pair chunks
NSC = S // 512          # 4 seq chunks of 512
NSO = S // P            # 16 seq chunks of 128
NKC = S // P            # 16 key chunks of 128

_cache = {}


def _build():
    """Build + compile the per-core Bass program (same program on all cores)."""
    import concourse.bass as bass
    import concourse.bacc as bacc
    import concourse.mybir as mybir
    import concourse.tile as tile
    from contextlib import ExitStack

    f32 = mybir.dt.float32
    bf16 = mybir.dt.bfloat16
    AF = mybir.ActivationFunctionType
    MUL = mybir.AluOpType.mult

    nc = bacc.Bacc("TRN2", target_bir_lowering=False, debug=False,
                   num_devices=NCORES)

    qT = nc.dram_tensor("qT", [D, S], bf16, kind="ExternalInput").ap()
    kT = nc.dram_tensor("kT", [D, S], bf16, kind="ExternalInput").ap()
    vT = nc.dram_tensor("vT", [D, S], bf16, kind="ExternalInput").ap()
    wq = nc.dram_tensor("wq", [D, PROJ], bf16, kind="ExternalInput").ap()
    wk = nc.dram_tensor("wk", [D, PROJ], bf16, kind="ExternalInput").ap()
    wv = nc.dram_tensor("wv", [D, PROJ], bf16, kind="ExternalInput").ap()
    wo = nc.dram_tensor("wo", [PROJ, D], bf16, kind="ExternalInput").ap()
    bq2 = nc.dram_tensor("bq2", [P, NPC], f32, kind="ExternalInput").ap()
    bk2 = nc.dram_tensor("bk2", [P, NPC], f32, kind="ExternalInput").ap()
    mk = nc.dram_tensor("mk", [P, NSO], f32, kind="ExternalInput").ap()
    sel = nc.dram_tensor("sel", [P, 128], bf16, kind="ExternalInput").ap()
    out = nc.dram_tensor("out", [S, D], f32, kind="ExternalOutput").ap()

    with tile.TileContext(nc) as tc, ExitStack() as ctx:
        cpool = ctx.enter_context(tc.tile_pool(name="const", bufs=1))
        sel_sb = cpool.tile([P, 128], bf16)
        nc.sync.dma_start(sel_sb[:], sel)
        bq_sb = cpool.tile([P, NPC], f32)
        nc.sync.dma_start(bq_sb[:], bq2)
        bk_sb = cpool.tile([P, NPC], f32)
        nc.sync.dma_start(bk_sb[:], bk2)
        mk_sb = cpool.tile([P, NSO], f32)
        nc.sync.dma_start(mk_sb[:], mk)
        # l values land in rows {0, 32}; all other rows must be finite (the
        # sel matmul's zero rows nullify them) -- one startup memset.
        Lsb = cpool.tile([P, 1024], bf16)
        nc.gpsimd.memset(Lsb[:], 0.0)
        ones8 = cpool.tile([P, HPC], bf16)
        nc.gpsimd.memset(ones8[:], 1.0)

        # Long-lived SBUF result tensors.
        respool = ctx.enter_context(tc.tile_pool(name="res", bufs=1))
        # Q^T pair-stacked: rows 0-63 = head 2*pr dims, rows 64-127 = head
        # 2*pr+1 dims.  K^T stored per head on the full 128-partition
        # contraction range: even heads carry data in rows 0-63 (rows 64-127
        # zero), odd heads in rows 64-127 (rows 0-63 zero), so the score
        # matmul is a full-array standard-mode matmul (no PE tiling modes --
        # partial-array matmuls keep the PE clock gate throttled).
        QT_sb = respool.tile([P, NPC, S], bf16)
        KT_sb = respool.tile([P, HPC, S], bf16)
        nc.gpsimd.memset(KT_sb[:], 0.0)
        # V with an interleaved mask column per head: head h occupies
        # cols [h*65, h*65+64) and col h*65+64 == mask (the masked softmax
        # denominator rides the AV matmul as output partition 64).  The mask
        # columns are filled on-device from mk (no 2MB strided DMA).
        V_sb = respool.tile([P, NSO, HPC * (DK + 1)], bf16)
        for so in range(NSO):
            nc.vector.tensor_scalar_mul(
                V_sb[:, so, :].rearrange("p (h w) -> p h w", w=DK + 1)[:, :, DK],
                ones8[:], mk_sb[:, so:so + 1])
        AT_sb = respool.tile([P, NPC, S], bf16)   # normalized A^T

        wopool = ctx.enter_context(tc.tile_pool(name="wo", bufs=1))
        wo_sb = wopool.tile([P, NPC, D], bf16)

        npool = ctx.enter_context(tc.tile_pool(name="norm", bufs=4))
        rcpool = ctx.enter_context(tc.tile_pool(name="rc", bufs=2))
        epool = ctx.enter_context(tc.tile_pool(name="expS", bufs=5))
        opool = ctx.enter_context(tc.tile_pool(name="ostage", bufs=4))

        # ---------------- Phase A: projections ----------------
        with ExitStack() as ctxA:
            wpool = ctxA.enter_context(tc.tile_pool(name="w", bufs=2))
            apool = ctxA.enter_context(tc.tile_pool(name="actT", bufs=2))
            psA = ctxA.enter_context(
                tc.tile_pool(name="psA", bufs=4, space="PSUM"))

            for w_hbm, x_hbm, bias_sb, dst in (
                (wq, qT, bq_sb, QT_sb),
                (wk, kT, bk_sb, KT_sb),
            ):
                w_sb = wpool.tile([P, NDM, PROJ], bf16, tag="w", name="w_sb")
                nc.sync.dma_start(w_sb[:], w_hbm.rearrange("(o p) n -> p o n", p=P))
                for sc in range(NSC):
                    a_sb = apool.tile([P, NDM, 512], bf16, tag="a", name="a_sb")
                    nc.sync.dma_start(
                        a_sb[:],
                        x_hbm.rearrange("(o p) s -> p o s", p=P)[
                            :, :, sc * 512:(sc + 1) * 512],
                    )
                    for pc in range(NPC):
                        ps = psA.tile([P, 512], f32, tag="pp", name="psa")
                        for dc in range(NDM):
                            nc.tensor.matmul(
                                ps,
                                lhsT=w_sb[:, dc, pc * P:(pc + 1) * P],
                                rhs=a_sb[:, dc, :],
                                start=(dc == 0), stop=(dc == NDM - 1),
                            )
                        if dst is QT_sb:
                            nc.vector.tensor_scalar_add(
                                dst[:, pc, sc * 512:(sc + 1) * 512], ps,
                                bias_sb[:, pc:pc + 1])
                        else:
                            for half in range(2):
                                lo = half * 64
                                nc.vector.tensor_scalar_add(
                                    KT_sb[lo:lo + 64, 2 * pc + half,
                                          sc * 512:(sc + 1) * 512],
                                    ps[lo:lo + 64, :],
                                    bias_sb[lo:lo + 64, pc:pc + 1])

            # V: out[seq-chunk(128), proj(512)] = vT^T @ Wv, scaled by mask
            w_sb = wpool.tile([P, NDM, PROJ], bf16, tag="w", name="w_sb")
            nc.sync.dma_start(w_sb[:], wv.rearrange("(o p) n -> p o n", p=P))
            # wo is needed only in phase C; queue it behind the V weights.
            nc.sync.dma_start(wo_sb[:], wo.rearrange("(o p) n -> p o n", p=P))
            for sc in range(NSC):
                a_sb = apool.tile([P, NDM, 512], bf16, tag="a", name="a_sb")
                nc.sync.dma_start(
                    a_sb[:],
                    vT.rearrange("(o p) s -> p o s", p=P)[
                        :, :, sc * 512:(sc + 1) * 512],
                )
                for so4 in range(4):
                    so = sc * 4 + so4
                    ps = psA.tile([P, 512], f32, tag="pp", name="psa")
                    for dc in range(NDM):
                        nc.tensor.matmul(
                            ps,
                            lhsT=a_sb[:, dc, so4 * P:(so4 + 1) * P],
                            rhs=w_sb[:, dc, :],
                            start=(dc == 0), stop=(dc == NDM - 1),
                        )
                    nc.vector.tensor_scalar_mul(
                        V_sb[:, so, :].rearrange(
                            "p (h w) -> p h w", w=DK + 1)[:, :, 0:DK],
                        ps.rearrange("p (h w) -> p h w", w=DK),
                        mk_sb[:, so:so + 1])

        # ---------------- Phases B + C, software pipelined ----------------
        with ExitStack() as ctxB:
            psS = ctxB.enter_context(
                tc.tile_pool(name="psS", bufs=2, space="PSUM"))
            psAcc = ctxB.enter_context(
                tc.tile_pool(name="psAcc", bufs=1, space="PSUM"))
            psX = ctxB.enter_context(
                tc.tile_pool(name="psX", bufs=2, space="PSUM"))

            def emit_c_chunk(qc, j):
                """Output projection for chunk j of query group qc."""
                so = qc * 8 + j // 2
                oc = j % 2
                ps = psX.tile([P, 512], f32, tag="px", name="psx")
                for pc in range(NPC):
                    nc.tensor.matmul(
                        ps,
                        lhsT=AT_sb[:, pc, so * P:(so + 1) * P],
                        rhs=wo_sb[:, pc, oc * 512:(oc + 1) * 512],
                        start=(pc == 0), stop=(pc == NPC - 1),
                    )
                ost = opool.tile([P, 512], f32, tag="o", name="ost")
                nc.vector.tensor_copy(ost, ps)
                nc.sync.dma_start(
                    out[so * P:(so + 1) * P, oc * 512:(oc + 1) * 512], ost)

            def pair_tail(pr, qc, a0, a1, final=False):
                """Broadcast 1/l and normalize both heads' A^T (background)."""
                bc = psS.tile([P, 1024], f32, tag="s", name="sp")
                for sub in range(2):
                    nc.tensor.matmul(
                        bc[:, sub * 512:(sub + 1) * 512],
                        lhsT=sel_sb[:],
                        rhs=Lsb[:, sub * 512:(sub + 1) * 512],
                        start=True, stop=True,
                    )
                rc = rcpool.tile([P, 1024], f32, tag="rc", name="rc")
                if final:
                    # Drain path: the score buffers are dead, so read the
                    # reciprocal straight out of PSUM.
                    nc.vector.reciprocal(rc, bc)
                else:
                    # Stage bc out of PSUM first (frees the borrowed score
                    # buffer in ~1.2us), then run the slow iterative
                    # reciprocal entirely in SBUF where it blocks nothing.
                    bcs = rcpool.tile([P, 1024], f32, tag="bcs", name="bcs")
                    nc.vector.tensor_copy(bcs, bc)
                    nc.vector.reciprocal(rc, bcs)
                for hi, at in ((0, a0), (1, a1)):
                    rows = slice(hi * 64, (hi + 1) * 64)
                    nc.vector.tensor_tensor(
                        AT_sb[rows, pr, qc * 1024:(qc + 1) * 1024],
                        at[rows, :], rc[rows, :], MUL)

            atn = [None, None]
            pending = []        # deferred PE/DVE work, emitted inside blocks
            cqueue = []         # deferred phase-C chunks
            carry = None        # (h, avs, es, hi) AV(14..15)+tail of prev block

            def emit_av(h, avs, es, kc, start, stop):
                for sub in range(2):
                    nc.tensor.matmul(
                        avs[0:DK + 1, sub * 512:(sub + 1) * 512],
                        lhsT=V_sb[:, kc, h * (DK + 1):(h + 1) * (DK + 1)],
                        rhs=es[kc][:, sub * 512:(sub + 1) * 512],
                        start=start, stop=stop,
                    )

            def emit_block_tail(h, avs, es, hi, final=False):
                """Last two AVs + the PSUM evacuation copies for a block."""
                emit_av(h, avs, es, NKC - 2, False, False)
                emit_av(h, avs, es, NKC - 1, False, True)
                if final:
                    # scalar engine is idle at the drain; let it handle the
                    # l row so the DVE can go straight to the reciprocal
                    nc.scalar.copy(Lsb[hi * 32:hi * 32 + 1, :],
                                   avs[DK:DK + 1, :])
                else:
                    nc.vector.tensor_copy(
                        Lsb[hi * 32:hi * 32 + 1, :], avs[DK:DK + 1, :])
                # Stage A^T into the partition range matching this head's rc
                # rows (all-SBUF DVE ops need identical partition windows;
                # PSUM sources are exempt).
                at = npool.tile([P, 1024], f32, tag="at", name="at")
                nc.vector.tensor_copy(
                    at[hi * 64:(hi + 1) * 64, :], avs[0:64, :])
                atn[hi] = at

            blocks = [(qc, pr, hi)
                      for qc in range(2) for pr in range(NPC) for hi in range(2)]
            for qc, pr, hi in blocks:
                h = 2 * pr + hi
                es = [None] * NKC
                avs = None
                for kc in range(NKC):
                    sp = psS.tile([P, 1024], f32, tag="s", name="sp")
                    for sub in range(2):
                        nc.tensor.matmul(
                            sp[:, sub * 512:(sub + 1) * 512],
                            lhsT=KT_sb[:, h, kc * P:(kc + 1) * P],
                            rhs=QT_sb[:, pr,
                                      qc * 1024 + sub * 512:
                                      qc * 1024 + (sub + 1) * 512],
                            start=True, stop=True,
                        )
                    e = epool.tile([P, 1024], bf16, tag="e", name="e")
                    nc.scalar.activation(
                        e, sp, AF.Exp, scale=float(1.0 / np.sqrt(DK)))
                    es[kc] = e
                    if kc == 1 and carry is not None:
                        # previous block's last AVs + PSUM evacuation run
                        # under this block's first scores/exps
                        emit_block_tail(*carry)
                        carry = None
                        if hi == 0:
                            pending.append(
                                (lambda pr2=pr - 1 if pr else NPC - 1,
                                 qc2=qc if pr else qc - 1,
                                 a0=atn[0], a1=atn[1]:
                                 pair_tail(pr2, qc2, a0, a1)))
                    if kc == 3:
                        # this block's AV accumulator (allocated after the
                        # previous block's evacuation copies for clean WAR)
                        avs = psAcc.tile([P, 1024], f32, tag="av", name="avs")
                        emit_av(h, avs, es, 0, True, False)
                        emit_av(h, avs, es, 1, False, False)
                    if kc == 4 and pending:
                        # previous pair's normalization: bc matmul +
                        # reciprocal + scale, all off the critical path
                        pending.pop(0)()
                    if kc in (6, 10, 13) and cqueue:
                        tc.no_sync_barrier()   # pin C chunks at this point
                        cqueue.pop(0)()
                    if kc > 3:
                        emit_av(h, avs, es, kc - 2, False, False)
                if (qc, pr, hi) == (1, 0, 1):
                    # queue the first query group's output projection; it is
                    # consumed (fenced) inside the remaining qc=1 blocks
                    cqueue.extend(
                        [(lambda j=j: emit_c_chunk(0, j)) for j in range(16)])
                carry = (h, avs, es, hi)

            # drain: last block tail, last pair's normalization, final outputs
            emit_block_tail(*carry, final=True)
            pending.append(
                (lambda a0=atn[0], a1=atn[1]:
                 pair_tail(NPC - 1, 1, a0, a1, final=True)))
            while pending:
                pending.pop(0)()
            while cqueue:       # any phase-C chunks not consumed in-loop
                tc.no_sync_barrier()
                cqueue.pop(0)()
            for j in range(16):
                emit_c_chunk(1, j)

    nc.compile()
    return nc


def _get_nc():
    if "nc" not in _cache:
        _cache["nc"] = _build()
    return _cache["nc"]


def make_in_maps(q, k, v, mask, Wq, bq, Wk, bk, Wv, bv, Wo, bo):
    """Host-side sharding: slice/transpose the full inputs per core."""
    import ml_dtypes
    f = np.float32
    bf = ml_dtypes.bfloat16
    q = np.asarray(q, dtype=f)
    k = np.asarray(k, dtype=f)
    v = np.asarray(v, dtype=f)
    Wq = np.asarray(Wq, dtype=f)
    Wk = np.asarray(Wk, dtype=f)
    Wv = np.asarray(Wv, dtype=f)
    Wo = np.asarray(Wo, dtype=f)
    bq = np.asarray(bq, dtype=f)
    bk = np.asarray(bk, dtype=f)
    mask = np.asarray(mask)

    sel = np.zeros((P, 128), dtype=f)
    sel[0, 0:64] = 1.0
    sel[32, 64:128] = 1.0

    in_maps = []
    for c in range(NCORES):
        b, hg = divmod(c, 2)
        cols = slice(hg * PROJ, (hg + 1) * PROJ)
        mvals = (mask[b, 0, 0, :] != 0).astype(f)            # [S]
        mk2 = np.ascontiguousarray(mvals.reshape(NSO, P).T)  # [P, NSO]
        in_maps.append({
            "qT": np.ascontiguousarray(q[b].T).astype(bf),
            "kT": np.ascontiguousarray(k[b].T).astype(bf),
            "vT": np.ascontiguousarray(v[b].T).astype(bf),
            "wq": np.ascontiguousarray(Wq[:, cols]).astype(bf),
            "wk": np.ascontiguousarray(Wk[:, cols]).astype(bf),
            "wv": np.ascontiguousarray(Wv[:, cols]).astype(bf),
            "wo": np.ascontiguousarray(Wo[cols, :]).astype(bf),
            "bq2": np.ascontiguousarray(bq[cols].reshape(NPC, P).T),
            "bk2": np.ascontiguousarray(bk[cols].reshape(NPC, P).T),
            "mk": mk2,
            "sel": sel.astype(bf),
        })
    return in_maps


def combine_outputs(parts, Wv_bv_Wo_bo):
    """Host-side unshard: sum the two head-group partials per batch, add the
    folded bias bv @ Wo + bo."""
    bv, Wo, bo = Wv_bv_Wo_bo
    bo_eff = (np.asarray(bv, np.float32) @ np.asarray(Wo, np.float32)
              + np.asarray(bo, np.float32))
    out = np.empty((B, S, D), dtype=np.float32)
    for b in range(B):
        out[b] = parts[2 * b] + parts[2 * b + 1] + bo_eff
    return out


def _install_axon_ntff_hook():
    """The agent image's antenv lacks axon_hooks; synthesize it and register
    the ctypes NTFF profile hook from trn_boot so trace=True works."""
    import sys
    import types
    if "antenv.axon_hooks" in sys.modules:
        return
    try:
        from trn_agent_boot.trn_boot import _ntff_profile_via_ctypes
        hook = _ntff_profile_via_ctypes("/opt/axon/libaxon_pjrt.so")
    except Exception:
        hook = None
    mod = types.ModuleType("antenv.axon_hooks")
    mod._hook = hook
    mod.get_axon_ntff_profile_hook = lambda: mod._hook
    mod.set_axon_ntff_profile_hook = lambda h: setattr(mod, "_hook", h)
    sys.modules["antenv.axon_hooks"] = mod
    # upload_artifacts wants a fish bucket; keep artifacts local instead.
    import concourse.bass_utils as bu
    bu.upload_artifacts = lambda tmpdir: str(tmpdir)


def kernel(q, k, v, mask, Wq, bq, Wk, bk, Wv, bv, Wo, bo):
    from concourse.bass_utils import run_bass_kernel_spmd

    nc = _get_nc()
    in_maps = make_in_maps(q, k, v, mask, Wq, bq, Wk, bk, Wv, bv, Wo, bo)
    trace = bool(int(os.environ.get("KERNEL_TRACE", "0")))
    if trace:
        try:
            _install_axon_ntff_hook()
        except Exception:
            trace = False
    try:
        res = run_bass_kernel_spmd(
            nc, in_maps, list(range(NCORES)), trace=trace,
            tmpdir=os.environ.get("KERNEL_TRACE_DIR") or None)
    except Exception:
        if not trace:
            raise
        # Trace machinery failed; rerun without it so results still flow.
        res = run_bass_kernel_spmd(nc, in_maps, list(range(NCORES)), trace=False)
    _cache["last_result"] = res
    parts = [res.results[c]["out"] for c in range(NCORES)]
    return combine_outputs(parts, (bv, Wo, bo))
